# revision 4
# baseline (speedup 1.0000x reference)
"""MultiHeadSimilarity kernel for 8 Trainium2 NeuronCores.

Reference computation (per batch b):
    Q = wq @ x[b];  K = wk @ y[b]                       (channel-mixing matmuls)
    per head h (d=64):  A = relu(Qh^T Kh) * scale, masked by xy_mask
    C = A @ Kh^T, normalized per-row by 1/max(sum(mask, y), 1)
    out = wo @ (0.5 * (Q + C))

Sharding: data-parallel over batch; 16 batches / 8 cores = 2 per core.
Weights replicated. No cross-core communication.

Device algorithm (fp16 + fp8-DoubleRow attention internals):
  - Q projection fp16 (accuracy: Q feeds the output directly).
  - K / KT projections in fp8 DoubleRow (y8 + wk8 inputs, 2 k-tiles per
    instruction = contraction 256): K drains to f16 (A stationary),
    KT drains to f8 (C stationary).
  - A^T per head stays fp16 (contraction d=64; DR gives no win there).
    Consecutive A matmuls alternate head row-quadrants so weight loads
    overlap streaming.
  - relu+mask quantizes straight to fp8: either one fused DVE
    scalar_tensor_tensor (PSUM f32 -> f8) or ACT relu->f8 + GPSIMD f8
    tensor_tensor, statically balanced across the three engines.
  - C matmul in fp8 DoubleRow: lhsT = KT8 head-pair block [128y, 2yt, 128ch]
    (M=128, half the rows garbage), rhs = Am8 [128y, 2yt, 512x]; two PSUM
    tiles per head pair; contraction 256 y per instruction = 2x fp16.
  - n_el row counts and 1/(8*max(n,1)) are computed on the HOST (mask is
    host-visible); inv row DMA'd and partition-broadcast on GPSIMD.
  - 0.5 is folded into woT on the host; output projection fp16.
"""
import sys

if "/opt/trn_rl_repo" not in sys.path:
    sys.path.insert(0, "/opt/trn_rl_repo")

import numpy as np

import concourse.tile as tile
from concourse import bacc, mybir
from concourse.bass_utils import run_bass_kernel_spmd

F16 = mybir.dt.float16
F32 = mybir.dt.float32
F8 = mybir.dt.float8e4
AL = mybir.AluOpType
RELU = mybir.ActivationFunctionType.Relu
DR = mybir.MatmulPerfMode.DoubleRow

N_CORES = 8
B, U, LX, LY, H, D = 16, 512, 1024, 1024, 8, 64
BPC = B // N_CORES          # batches per core
KB = U // 128               # 4  tiles over channels
HP = H // 2                 # 4  head pairs
YT = LY // 128              # 8  y tiles
XH = LX // 512              # 2  x halves
INV_SCALE = float(D) ** 0.5  # 8.0; attention scale 1/8 folded into host inv

TRACE = False
_CACHE = {}

# measured per-op costs (ns) for the static 3-engine balancer
C_DVE_STT = 1223.0   # fused relu+mask+quant fd1024, PSUM f32 in
C_ACT_RELU = 1114.0  # ACT relu fd1024 PSUM -> SBUF f8
C_GP_TT = 2007.0     # GPSIMD f8 mask mult fd1024 SBUF
C_DVE_ET = 670.0     # TT fd512 PSUM f32 x invb
C_DVE_EADD = 400.0   # TT fd512 f16 SBUF 2x
C_ACT_CP1024 = 1137.0
C_ACT_CP512 = 710.0
C_DVE_CP1024 = 1200.0  # PSUM f32 source, 1x
C_DVE_CP512 = 670.0


class Bal3:
    """Static greedy balancer across DVE / ACT / GP."""

    def __init__(self, nc):
        self.nc = nc
        self.t = {"v": 0.0, "s": 0.0, "g": 0.0}

    def add(self, eng, ns):
        self.t[eng] += ns

    def copy(self, dst, src, fd):
        """PSUM->SBUF drain on DVE or ACT, whichever is less loaded."""
        dve = C_DVE_CP1024 if fd >= 1024 else C_DVE_CP512
        act = C_ACT_CP1024 if fd >= 1024 else C_ACT_CP512
        if self.t["v"] + dve <= self.t["s"] + act:
            self.t["v"] += dve
            self.nc.vector.tensor_copy(dst, src)
        else:
            self.t["s"] += act
            self.nc.scalar.copy(dst, src)

    def relu_mask_pair(self, out8, a_ps, mtf2, tmp_pool, name):
        """out8[(128,2,512) f8] = relu(a_ps) * mtf2 (two y-tiles, one head).

        option v: fused DVE scalar_tensor_tensor
        option sg: ACT relu -> f8 + GPSIMD f8 multiply
        """
        v_end = self.t["v"] + C_DVE_STT
        sg_end = max(self.t["s"] + C_ACT_RELU, self.t["g"] + C_GP_TT)
        if v_end <= sg_end:
            self.t["v"] += C_DVE_STT
            self.nc.vector.scalar_tensor_tensor(out8[:], a_ps[:], 0.0, mtf2,
                                                AL.max, AL.mult)
        else:
            self.t["s"] += C_ACT_RELU
            self.t["g"] += C_GP_TT
            ra = tmp_pool.tile([128, 2, 512], F8, tag="ra", name=name)
            self.nc.scalar.activation(ra[:], a_ps[:], RELU)
            self.nc.gpsimd.tensor_tensor(out8[:], ra[:], mtf2, AL.mult)


def _build():
    nc = bacc.Bacc("TRN2", target_bir_lowering=False, debug=False,
                   num_devices=N_CORES)
    x_e = nc.dram_tensor("x", [BPC, U, LX], F16, kind="ExternalInput")
    y_e = nc.dram_tensor("y", [BPC, U, LY], F8, kind="ExternalInput")
    mt_e = nc.dram_tensor("mt", [BPC, LY, LX], F8, kind="ExternalInput")
    inv_e = nc.dram_tensor("inv", [BPC, LX], F32, kind="ExternalInput")
    wq_e = nc.dram_tensor("wq_t", [U, U], F16, kind="ExternalInput")
    wo_e = nc.dram_tensor("wo_t", [U, U], F16, kind="ExternalInput")
    wk8_e = nc.dram_tensor("wk8_t", [U, U], F8, kind="ExternalInput")
    o_e = nc.dram_tensor("o", [BPC, U, LX], F32, kind="ExternalOutput")

    with tile.TileContext(nc) as tc:
        _emit(nc, tc, x_e, y_e, mt_e, inv_e, wq_e, wo_e, wk8_e, o_e)
    nc.compile()
    return nc


def _emit(nc, tc, x_e, y_e, mt_e, inv_e, wq_e, wo_e, wk8_e, o_e):
    import contextlib
    bal = Bal3(nc)
    ctx = contextlib.ExitStack()
    with ctx:
        wp = ctx.enter_context(tc.tile_pool(name="wp", bufs=1))
        io = ctx.enter_context(tc.tile_pool(name="io", bufs=2))
        pr = ctx.enter_context(tc.tile_pool(name="pr", bufs=2))
        sm = ctx.enter_context(tc.tile_pool(name="sm", bufs=2))
        amp = ctx.enter_context(tc.tile_pool(name="amp", bufs=4))
        osp = ctx.enter_context(tc.tile_pool(name="osp", bufs=3))
        pa = ctx.enter_context(tc.tile_pool(name="pa", bufs=2, space="PSUM"))
        pc = ctx.enter_context(tc.tile_pool(name="pc", bufs=4, space="PSUM"))

        # weights, loaded once
        WQT = wp.tile([128, KB, U], F16, tag="wqt")
        WOT = wp.tile([128, KB, U], F16, tag="wot")
        WK8 = wp.tile([128, KB, U], F8, tag="wk8")
        nc.scalar.dma_start(
            WQT[:], wq_e.ap().rearrange("(k p) o -> p k o", p=128))
        nc.scalar.dma_start(
            WOT[:], wo_e.ap().rearrange("(k p) o -> p k o", p=128))
        nc.scalar.dma_start(
            WK8[:], wk8_e.ap().rearrange("(k p) o -> p k o", p=128))

        for b in range(BPC):
            # ---- input loads ----
            X = io.tile([128, KB, LX], F16, tag="x", name=f"x{b}")
            Y8 = io.tile([128, KB, LY], F8, tag="y8", name=f"y8{b}")
            for k in range(KB):
                nc.sync.dma_start(X[:, k, :], x_e.ap()[b, k * 128:(k + 1) * 128, :])
            for k in range(KB):
                nc.gpsimd.dma_start(Y8[:, k, :], y_e.ap()[b, k * 128:(k + 1) * 128, :])
            MTF8 = io.tile([128, YT, LX], F8, tag="mtf8", name=f"mtf8{b}")
            for t in range(YT):
                (nc.sync if t % 2 == 0 else nc.gpsimd).dma_start(
                    MTF8[:, t, :], mt_e.ap()[b, t * 128:(t + 1) * 128, :])
            invr = sm.tile([1, LX], F32, tag="invr", name=f"invr{b}")
            nc.sync.dma_start(invr[:], inv_e.ap()[b:b + 1, :])
            invb = sm.tile([128, LX], F32, tag="invb", name=f"invb{b}")
            nc.gpsimd.partition_broadcast(invb[:], invr[:])

            # ---- Q projection: fp16 ----
            Q = pr.tile([128, KB, LX], F16, tag="q", name=f"q{b}")
            for m in range(KB):
                ps = pa.tile([128, 2, 512], F32, tag="a", name=f"pq{b}_{m}")
                for k in range(KB):
                    for n in range(XH):
                        nc.tensor.matmul(
                            ps[:, n, :], WQT[:, k, m * 128:(m + 1) * 128],
                            X[:, k, n * 512:(n + 1) * 512],
                            start=(k == 0), stop=(k == KB - 1))
                bal.copy(Q[:, m, :], ps[:], 1024)

            # ---- K projection: fp8 DoubleRow -> K f16 ----
            K = pr.tile([128, KB, LY], F16, tag="k", name=f"k{b}")
            for m in range(KB):
                ps = pa.tile([128, 2, 512], F32, tag="a", name=f"pk{b}_{m}")
                for j in range(2):
                    for n in range(XH):
                        nc.tensor.matmul(
                            ps[:, n, :],
                            WK8[:, 2 * j:2 * j + 2, m * 128:(m + 1) * 128],
                            Y8[:, 2 * j:2 * j + 2, n * 512:(n + 1) * 512],
                            start=(j == 0), stop=(j == 1), perf_mode=DR)
                bal.copy(K[:, m, :], ps[:], 1024)

            # ---- KT projection: fp8 DoubleRow -> KT8 f8 ----
            # KT[y, ch] = sum_u y8[u, y] * wk8T[u, ch]
            KT8 = pr.tile([128, YT, U], F8, tag="kt8", name=f"kt8{b}")
            for lt2 in range(YT // 2):
                ps = pa.tile([128, 2, 512], F32, tag="a", name=f"pkt{b}_{lt2}")
                for i in range(2):
                    lt = lt2 * 2 + i
                    for j in range(2):
                        nc.tensor.matmul(
                            ps[:, i, :],
                            Y8[:, 2 * j:2 * j + 2, lt * 128:(lt + 1) * 128],
                            WK8[:, 2 * j:2 * j + 2, :512],
                            start=(j == 0), stop=(j == 1), perf_mode=DR)
                bal.copy(KT8[:, lt2 * 2:lt2 * 2 + 2, :], ps[:], 1024)

            # ---- attention ----
            E = pr.tile([128, KB, LX], F16, tag="e", name=f"e{b}")
            for hp in range(HP):
                for xh in range(XH):
                    xs = slice(xh * 512, (xh + 1) * 512)
                    # two C psum tiles (per head), M=128, half rows garbage
                    C0 = pc.tile([128, 512], F32, tag="c", name=f"c0_{b}_{hp}_{xh}")
                    C1 = pc.tile([128, 512], F32, tag="c", name=f"c1_{b}_{hp}_{xh}")
                    for t in range(YT // 2):
                        # Am8 free layout: (ytile i, head j, x)
                        Am = amp.tile([128, 2, 2, 512], F8, tag="am", bufs=4,
                                      name=f"am_{b}_{hp}_{xh}_{t}")
                        for i in range(2):
                            yt = 2 * t + i
                            # one A psum tile per ytile, both heads (j slots),
                            # quadrant ping-pong on the 64-row stationaries
                            A = pa.tile([128, 2, 512], F32, tag="a",
                                        name=f"a_{b}_{hp}_{xh}_{yt}")
                            for j in range(2):
                                hs = slice(64 * j, 64 * (j + 1))
                                nc.tensor.matmul(
                                    A[:, j, :],
                                    K[hs, hp, yt * 128:(yt + 1) * 128],
                                    Q[hs, hp, xs], start=True, stop=True)
                            mtf_b = MTF8[:, yt, xs].unsqueeze(1).broadcast_to(
                                (128, 2, 512))
                            bal.relu_mask_pair(Am[:, i, :, :], A, mtf_b, amp,
                                               f"ra_{b}_{hp}_{xh}_{yt}")
                        ktp = KT8[:, 2 * t:2 * t + 2, hp * 128:(hp + 1) * 128]
                        for j, Cps in ((0, C0), (1, C1)):
                            nc.tensor.matmul(Cps[:], ktp, Am[:, :, j, :],
                                             start=(t == 0),
                                             stop=(t == YT // 2 - 1),
                                             perf_mode=DR)
                    # drains: head 2hp valid in C0 rows 0-63,
                    #         head 2hp+1 valid in C1 rows 64-127
                    for j, Cps in ((0, C0), (1, C1)):
                        hs = slice(64 * j, 64 * (j + 1))
                        Et = amp.tile([128, 512], F16, tag="et",
                                      name=f"et{b}_{hp}_{xh}_{j}")
                        nc.vector.tensor_tensor(Et[hs, :], Cps[hs, :],
                                                invb[hs, xs], AL.mult)
                        bal.add("v", C_DVE_ET)
                        nc.vector.tensor_tensor(E[hs, hp, xs], Et[hs, :],
                                                Q[hs, hp, xs], AL.add)
                        bal.add("v", C_DVE_EADD)

            # ---- output projection: fp16 ----
            for m in range(KB):
                ps = pa.tile([128, 2, 512], F32, tag="a", name=f"po{b}_{m}")
                for k in range(KB):
                    for n in range(XH):
                        nc.tensor.matmul(ps[:, n, :],
                                         WOT[:, k, m * 128:(m + 1) * 128],
                                         E[:, k, n * 512:(n + 1) * 512],
                                         start=(k == 0), stop=(k == KB - 1))
                oS = osp.tile([128, LX], F32, tag="os", name=f"os{b}_{m}")
                for n in range(XH):
                    bal.copy(oS[:, n * 512:(n + 1) * 512], ps[:, n, :], 512)
                    nc.sync.dma_start(
                        o_e.ap()[b, m * 128:(m + 1) * 128, n * 512:(n + 1) * 512],
                        oS[:, n * 512:(n + 1) * 512])


def _get_nc():
    if "nc" not in _CACHE:
        _CACHE["nc"] = _build()
    return _CACHE["nc"]


def kernel(x, y, xy_mask, wq, wk, wo):
    import ml_dtypes
    nc = _get_nc()
    xf = x.astype(np.float16)
    y8 = y.astype(ml_dtypes.float8_e4m3fn)
    mtt = np.ascontiguousarray(
        xy_mask.transpose(0, 2, 1)).astype(ml_dtypes.float8_e4m3fn)
    nel = np.maximum(xy_mask.sum(axis=2), 1).astype(np.float32)  # (B, LX)
    inv = (1.0 / (INV_SCALE * nel)).astype(np.float32)
    wqT = np.ascontiguousarray(wq.T).astype(np.float16)
    woT = np.ascontiguousarray((0.5 * wo).T).astype(np.float16)
    wk8T = np.ascontiguousarray(wk.T).astype(ml_dtypes.float8_e4m3fn)
    in_maps = [
        {"x": xf[c * BPC:(c + 1) * BPC], "y": y8[c * BPC:(c + 1) * BPC],
         "mt": mtt[c * BPC:(c + 1) * BPC], "inv": inv[c * BPC:(c + 1) * BPC],
         "wq_t": wqT, "wo_t": woT, "wk8_t": wk8T}
        for c in range(N_CORES)
    ]
    res = run_bass_kernel_spmd(nc, in_maps, list(range(N_CORES)), trace=TRACE)
    if TRACE:
        _CACHE["last_exec_time_ns"] = res.exec_time_ns
        _CACHE["last_profile_json"] = res.profile_json
    return np.concatenate([res.results[c]["o"] for c in range(N_CORES)], axis=0)


# revision 5
# speedup vs baseline: 1.1927x; 1.1927x over previous
"""MultiHeadSimilarity kernel for 8 Trainium2 NeuronCores.

Reference computation (per batch b):
    Q = wq @ x[b];  K = wk @ y[b]                       (channel-mixing matmuls)
    per head h (d=64):  A = relu(Qh^T Kh) * scale, masked by xy_mask
    C = A @ Kh^T, normalized per-row by 1/max(sum(mask, y), 1)
    out = wo @ (0.5 * (Q + C))

Sharding: data-parallel over batch; 16 batches / 8 cores = 2 per core.
Weights replicated. No cross-core communication.

Device algorithm (fp16 compute, fp32 PSUM):
  - Q = wqT.T @ x, K = wkT.T @ y (natural-layout fp16 matmuls).
  - KT (y on partitions, needed as the C-matmul stationary) comes from a
    DMA XBAR transpose of K — zero tensor-engine cycles (the baseline
    recomputed it as a second projection).
  - n_el row counts and inv = 1/(8*max(n,1)) are computed on the HOST
    (the mask is host-visible); inv is DMA'd and partition-broadcast.
  - A is computed transposed (y on partitions) per head pair; two heads
    pack the 128-wide PE array as 64-row groups (quadrant ping-pong hides
    weight loads). relu+mask fuses into one op, statically balanced over
    THREE engines: DVE fused scalar_tensor_tensor, or ACT relu + DVE
    multiply, or ACT relu + GPSIMD multiply. The mask is fp8 (halves its
    DMA; exact 0/1 values).
  - C accumulates two heads per PSUM bank (64-col groups); normalization
    and the E = 0.5(Q+C) merge ride DVE; 0.5 is folded into woT on host.
"""
import sys

if "/opt/trn_rl_repo" not in sys.path:
    sys.path.insert(0, "/opt/trn_rl_repo")

import numpy as np

import concourse.tile as tile
from concourse import bacc, mybir
from concourse.bass_utils import run_bass_kernel_spmd

F16 = mybir.dt.float16
F32 = mybir.dt.float32
F8 = mybir.dt.float8e4
AL = mybir.AluOpType
RELU = mybir.ActivationFunctionType.Relu

N_CORES = 8
B, U, LX, LY, H, D = 16, 512, 1024, 1024, 8, 64
BPC = B // N_CORES          # batches per core
KB = U // 128               # 4  k-tiles over channels
HP = H // 2                 # 4  head pairs
YT = LY // 128              # 8  y tiles
XH = LX // 512              # 2  x halves
INV_SCALE = float(D) ** 0.5  # 8.0; attention scale 1/8 folded into host inv

TRACE = False
_CACHE = {}

# measured per-op costs (ns) for the static 3-engine balancer
C_DVE_STT = 1223.0   # fused relu+mask fd1024, PSUM f32 in
C_ACT_RELU = 1114.0  # ACT relu fd1024 PSUM -> SBUF
C_DVE_TT2 = 665.0    # DVE f16 mask mult fd1024, 2x
C_GP_TT = 2030.0     # GPSIMD mask mult fd1024
C_ACT_CP1024 = 1137.0
C_ACT_CP512 = 710.0
C_DVE_CP1024 = 1200.0
C_DVE_CP512 = 670.0


class Bal3:
    """Static greedy balancer across DVE / ACT / GP."""

    def __init__(self, nc):
        self.nc = nc
        self.t = {"v": 0.0, "s": 0.0, "g": 0.0}

    def add(self, eng, ns):
        self.t[eng] += ns

    def copy(self, dst, src, fd):
        dve = C_DVE_CP1024 if fd >= 1024 else C_DVE_CP512
        act = C_ACT_CP1024 if fd >= 1024 else C_ACT_CP512
        if self.t["v"] + dve <= self.t["s"] + act:
            self.t["v"] += dve
            self.nc.vector.tensor_copy(dst, src)
        else:
            self.t["s"] += act
            self.nc.scalar.copy(dst, src)

    def relu_mask_pair(self, out, a_ps, mtf_b, tmp_pool, name):
        """out[(128,2,512) f16] = relu(a_ps) * mtf_b (two heads, one y-tile).

        option v:  fused DVE scalar_tensor_tensor (PSUM in)
        option sd: ACT relu -> f16 + DVE f16 multiply (2x)
        option sg: ACT relu -> f16 + GPSIMD multiply
        """
        v_end = self.t["v"] + C_DVE_STT
        sd_end = max(self.t["s"] + C_ACT_RELU, self.t["v"] + C_DVE_TT2)
        sg_end = max(self.t["s"] + C_ACT_RELU, self.t["g"] + C_GP_TT)
        best = min(v_end, sd_end, sg_end)
        if best == v_end:
            self.t["v"] += C_DVE_STT
            self.nc.vector.scalar_tensor_tensor(out[:], a_ps[:], 0.0, mtf_b,
                                                AL.max, AL.mult)
            return
        ra = tmp_pool.tile([128, 2, 512], F16, tag="ra", name=name)
        self.nc.scalar.activation(ra[:], a_ps[:], RELU)
        self.t["s"] += C_ACT_RELU
        if best == sd_end:
            self.t["v"] += C_DVE_TT2
            self.nc.vector.tensor_tensor(out[:], ra[:], mtf_b, AL.mult)
        else:
            self.t["g"] += C_GP_TT
            self.nc.gpsimd.tensor_tensor(out[:], ra[:], mtf_b, AL.mult)


def _build():
    nc = bacc.Bacc("TRN2", target_bir_lowering=False, debug=False,
                   num_devices=N_CORES)
    x_e = nc.dram_tensor("x", [BPC, U, LX], F16, kind="ExternalInput")
    y_e = nc.dram_tensor("y", [BPC, U, LY], F16, kind="ExternalInput")
    mt_e = nc.dram_tensor("mt", [BPC, LY, LX], F8, kind="ExternalInput")
    inv_e = nc.dram_tensor("inv", [BPC, LX], F32, kind="ExternalInput")
    w_all_e = nc.dram_tensor("w_all", [3, U, U], F16, kind="ExternalInput")
    o_e = nc.dram_tensor("o", [BPC, U, LX], F32, kind="ExternalOutput")

    with tile.TileContext(nc) as tc:
        _emit(nc, tc, x_e, y_e, mt_e, inv_e, w_all_e, o_e)
    nc.compile()
    return nc


def _emit(nc, tc, x_e, y_e, mt_e, inv_e, w_all_e, o_e):
    import contextlib
    bal = Bal3(nc)
    ctx = contextlib.ExitStack()
    with ctx:
        wp = ctx.enter_context(tc.tile_pool(name="wp", bufs=1))
        io = ctx.enter_context(tc.tile_pool(name="io", bufs=2))
        pr = ctx.enter_context(tc.tile_pool(name="pr", bufs=2))
        sm = ctx.enter_context(tc.tile_pool(name="sm", bufs=2))
        amp = ctx.enter_context(tc.tile_pool(name="amp", bufs=4))
        osp = ctx.enter_context(tc.tile_pool(name="osp", bufs=3))
        pa = ctx.enter_context(tc.tile_pool(name="pa", bufs=3, space="PSUM"))
        pc = ctx.enter_context(tc.tile_pool(name="pc", bufs=2, space="PSUM"))

        # weights, loaded once
        WQT = wp.tile([128, KB, U], F16, tag="wqt")
        WKT = wp.tile([128, KB, U], F16, tag="wkt")
        WOT = wp.tile([128, KB, U], F16, tag="wot")
        for wi, w_t in enumerate((WQT, WKT, WOT)):
            nc.scalar.dma_start(
                w_t[:], w_all_e.ap()[wi].rearrange("(k p) o -> p k o", p=128))

        for b in range(BPC):
            # ---- input loads ----
            X = io.tile([128, KB, LX], F16, tag="x", name=f"x{b}")
            Y = io.tile([128, KB, LY], F16, tag="y", name=f"y{b}")
            for k in range(KB):
                nc.sync.dma_start(X[:, k, :], x_e.ap()[b, k * 128:(k + 1) * 128, :])
            for k in range(KB):
                nc.gpsimd.dma_start(Y[:, k, :], y_e.ap()[b, k * 128:(k + 1) * 128, :])
            MTF8 = io.tile([128, YT, LX], F8, tag="mtf8", name=f"mtf8{b}")
            for t in range(YT):
                (nc.sync if t % 2 == 0 else nc.gpsimd).dma_start(
                    MTF8[:, t, :], mt_e.ap()[b, t * 128:(t + 1) * 128, :])
            invr = sm.tile([1, LX], F32, tag="invr", name=f"invr{b}")
            nc.sync.dma_start(invr[:], inv_e.ap()[b:b + 1, :])
            invb = sm.tile([128, LX], F32, tag="invb", name=f"invb{b}")
            nc.gpsimd.partition_broadcast(invb[:], invr[:])

            # ---- projections: Q = wqT.T @ x, K = wkT.T @ y ----
            Q = pr.tile([128, KB, LX], F16, tag="q", name=f"q{b}")
            K = pr.tile([128, KB, LY], F16, tag="k", name=f"k{b}")
            for w_t, src, dst in ((WQT, X, Q), (WKT, Y, K)):
                for m in range(KB):
                    ps = pa.tile([128, 2, 512], F32, tag="a",
                                 name=f"pj{b}_{dst.name}_{m}")
                    for k in range(KB):
                        for n in range(XH):
                            nc.tensor.matmul(
                                ps[:, n, :], w_t[:, k, m * 128:(m + 1) * 128],
                                src[:, k, n * 512:(n + 1) * 512],
                                start=(k == 0), stop=(k == KB - 1))
                    bal.copy(dst[:, m, :], ps[:], 1024)
            # KT via DMA XBAR transpose of K (no tensor-engine cycles)
            KT = pr.tile([128, YT, U], F16, tag="kt", name=f"kt{b}")
            for m in range(KB):
                nc.sync.dma_start_transpose(
                    KT[:, :, m * 128:(m + 1) * 128], K[:, m, :])

            # ---- attention ----
            E = pr.tile([128, KB, LX], F16, tag="e", name=f"e{b}")
            for hp in range(HP):
                for xh in range(XH):
                    xs = slice(xh * 512, (xh + 1) * 512)
                    # both heads accumulate into ONE bank: j0 at partitions
                    # 0-63 (col group 0), j1 at 64-127 (col group 64).
                    C = pc.tile([128, 512], F32, tag="c", name=f"c_{b}_{hp}_{xh}")
                    for yt in range(YT):
                        A = pa.tile([128, 2, 512], F32, tag="a",
                                    name=f"a_{b}_{hp}_{xh}_{yt}")
                        for j in range(2):
                            hs = slice(64 * j, 64 * (j + 1))
                            nc.tensor.matmul(
                                A[:, j, :], K[hs, hp, yt * 128:(yt + 1) * 128],
                                Q[hs, hp, xs], start=True, stop=True)
                        Am = amp.tile([128, 2, 512], F16, tag="am", bufs=6,
                                      name=f"am_{b}_{hp}_{xh}_{yt}")
                        mtf_b = MTF8[:, yt, xs].unsqueeze(1).broadcast_to(
                            (128, 2, 512))
                        bal.relu_mask_pair(Am, A, mtf_b, amp,
                                           f"ra_{b}_{hp}_{xh}_{yt}")
                        for j in range(2):
                            hs = slice(64 * j, 64 * (j + 1))
                            nc.tensor.matmul(
                                C[hs, :],
                                KT[:, yt, hp * 128 + 64 * j: hp * 128 + 64 * (j + 1)],
                                Am[:, j, :], start=(yt == 0), stop=(yt == YT - 1),
                                skip_group_check=True)
                    Et = amp.tile([128, 512], F16, tag="et", name=f"et_{b}_{hp}_{xh}")
                    nc.vector.tensor_tensor(Et[:], C[:], invb[:, xs], AL.mult)
                    bal.add("v", C_DVE_CP512)
                    nc.vector.tensor_tensor(E[:, hp, xs], Et[:], Q[:, hp, xs], AL.add)
                    bal.add("v", 400.0)

            # ---- output projection ----
            for m in range(KB):
                ps = pa.tile([128, 2, 512], F32, tag="a", name=f"po{b}_{m}")
                for k in range(KB):
                    for n in range(XH):
                        nc.tensor.matmul(ps[:, n, :],
                                         WOT[:, k, m * 128:(m + 1) * 128],
                                         E[:, k, n * 512:(n + 1) * 512],
                                         start=(k == 0), stop=(k == KB - 1))
                oS = osp.tile([128, LX], F32, tag="os", name=f"os{b}_{m}")
                for n in range(XH):
                    bal.copy(oS[:, n * 512:(n + 1) * 512], ps[:, n, :], 512)
                    nc.sync.dma_start(
                        o_e.ap()[b, m * 128:(m + 1) * 128, n * 512:(n + 1) * 512],
                        oS[:, n * 512:(n + 1) * 512])


def _get_nc():
    if "nc" not in _CACHE:
        _CACHE["nc"] = _build()
    return _CACHE["nc"]


def kernel(x, y, xy_mask, wq, wk, wo):
    import ml_dtypes
    nc = _get_nc()
    xf = x.astype(np.float16)
    yf = y.astype(np.float16)
    mtt = np.ascontiguousarray(
        xy_mask.transpose(0, 2, 1)).astype(ml_dtypes.float8_e4m3fn)
    nel = np.maximum(xy_mask.sum(axis=2), 1).astype(np.float32)  # (B, LX)
    inv = (1.0 / (INV_SCALE * nel)).astype(np.float32)
    w_all = np.stack([wq.T, wk.T, (0.5 * wo).T]).astype(np.float16)
    w_all = np.ascontiguousarray(w_all)
    in_maps = [
        {"x": xf[c * BPC:(c + 1) * BPC], "y": yf[c * BPC:(c + 1) * BPC],
         "mt": mtt[c * BPC:(c + 1) * BPC], "inv": inv[c * BPC:(c + 1) * BPC],
         "w_all": w_all}
        for c in range(N_CORES)
    ]
    res = run_bass_kernel_spmd(nc, in_maps, list(range(N_CORES)), trace=TRACE)
    if TRACE:
        _CACHE["last_exec_time_ns"] = res.exec_time_ns
        _CACHE["last_profile_json"] = res.profile_json
    return np.concatenate([res.results[c]["o"] for c in range(N_CORES)], axis=0)
